# revision 44
# baseline (speedup 1.0000x reference)
"""AttentionRPE kernel for 8 Trainium2 NeuronCores — v2 G^T design.

Math (per (b,s) row, T=128 targets, D=256, H=8 heads, DH=32, DR=32):
  q   = src @ Wsrc.T + bsrc                       [D]
  K'  = tgt @ Wk.T + rpe @ Rwk.T                  [T, D]
  att = softmax_h(q_h . K'_h / sqrt(DH))          [H, T]   (masked)
  out = (att @ V')_heads @ Wout.T + bout          [D]

Device formulation (per core, 128 rows, 8 blocks of 16 rows):
  * q-path folded ON HOST into per-row vectors qk[f, (s,h)] (f = 288
    tgt|rpe features).  logits[(s,h), t] = sum_f qk[f,(s,h)] tgtxT[f,t],
    computed 4-row-group col-tiled with the padding mask + off-window
    -1e30 folded in as 4 extra stationary rows (one-hot selector).
  * softmax WITHOUT max-subtraction (logits are O(10), exp is fp32-safe;
    -1e30 slots underflow to exactly 0).  exp -> bf16, den accumulated.
  * 4 PE window-transposes of exp give attT tiles whose off-diagonal
    (wrong-window) columns are EXACT zeros -> the 16 G^T matmuls
    (stationary attT strip, moving raw tgtx natural fp8) accumulate into
    ONE clean PSUM tile G^T[(j,h), f] with no garbage.
  * 1/den (softmax norm) is applied per-partition during the single
    G^T PSUM->SBUF copy (partitions ARE (j,h) there).
  * 3 PE transposes give G[f, (j,h)] -> gall; final out = sum_k
    gall-chunk^T @ wfx-chunk with host-folded wfx = (Wout_h @ Wvx_h).T.

Sharding: 1024 (b,s) rows split contiguously over 8 cores (128 each).
"""

import numpy as np
import ml_dtypes

import concourse.bass as bass
import concourse.bacc as bacc
import concourse.mybir as mybir
from concourse.tile import TileContext
from concourse.masks import make_identity
from concourse.bass_utils import run_bass_kernel_spmd

B, S, T, D = 2, 512, 128, 256
H, DH, DR = 8, 32, 32
DX = D + DR          # 288 = tgt|rpe feature dim
DOUT = D
NCORES = 8
BS = B * S           # 1024 total rows
SC = BS // NCORES    # 128 rows per core
NBLK = SC // 16      # 8 blocks of 16 rows

F32 = mybir.dt.float32
BF16 = mybir.dt.bfloat16
F8E3 = mybir.dt.float8e3
NPBF16 = np.dtype(ml_dtypes.bfloat16)
NPF8E3 = np.dtype(ml_dtypes.float8_e3m4)

AX = mybir.AxisListType
ALU = mybir.AluOpType
ACTF = mybir.ActivationFunctionType


def build(sc=SC):
    assert sc % 16 == 0
    nblk = sc // 16
    nc = bacc.Bacc()

    txt_d = nc.dram_tensor("txt", [nblk, 128, 4096], BF16, kind="ExternalInput")
    txn_d = nc.dram_tensor("txn", [nblk, 128, 4608], F8E3, kind="ExternalInput")
    tx2_d = nc.dram_tensor("tx2", [nblk, 36, 2048], BF16, kind="ExternalInput")
    qk01_d = nc.dram_tensor("qk01", [128, 2 * sc * H], BF16, kind="ExternalInput")
    qk2_d = nc.dram_tensor("qk2", [36, sc * H], BF16, kind="ExternalInput")
    wfxa_d = nc.dram_tensor("wfxa", [128, 16, DOUT], BF16, kind="ExternalInput")
    wfxb_d = nc.dram_tensor("wfxb", [32, 8, DOUT], BF16, kind="ExternalInput")
    obias_d = nc.dram_tensor("obias", [sc, DOUT], F32, kind="ExternalInput")
    rmask_d = nc.dram_tensor("rmask", [sc, 1], F32, kind="ExternalInput")
    out_d = nc.dram_tensor("out", [sc, DOUT], F32, kind="ExternalOutput")

    with TileContext(nc) as tc:
        with (
            tc.tile_pool(name="const", bufs=1) as cp,
            tc.tile_pool(name="txtp", bufs=8) as txtp,
            tc.tile_pool(name="txnp", bufs=8) as txnp,
            tc.tile_pool(name="tx2p", bufs=8) as tx2p,
            tc.tile_pool(name="attnp", bufs=2) as attnp,
            tc.tile_pool(name="smallp", bufs=2) as smallp,
            tc.tile_pool(name="ps_l", bufs=2, space="PSUM") as ps_l,
            tc.tile_pool(name="ps_g", bufs=1, space="PSUM") as ps_g,
            tc.tile_pool(name="ps_t", bufs=1, space="PSUM") as ps_t,
        ):
            # ---------------- constants ----------------
            qk01 = cp.tile([128, 2 * sc * H], BF16, name="qk01")
            sh = sc * H
            qk2 = cp.tile([36, sc * H], BF16, name="qk2")
            eyeb = cp.tile([128, 128], BF16, name="eyeb")
            make_identity(nc, eyeb)
            obias = cp.tile([sc, DOUT], F32, name="obias")
            rmask = cp.tile([sc, 1], F32, name="rmask")
            wfxa = cp.tile([128, 16, DOUT], BF16, name="wfxa")
            wfxb = cp.tile([32, 8, DOUT], BF16, name="wfxb")
            gall = cp.tile([128, 3, 8, sc], BF16, name="gall")

            # --- all stream tiles pre-allocated (bufs = nblk) ---
            txts = [txtp.tile([128, 4096], BF16, tag="txt", name=f"txt{b}")
                    for b in range(nblk)]
            txns = [txnp.tile([128, 4608], F8E3, tag="txn", name=f"txn{b}")
                    for b in range(nblk)]
            t2bs = [tx2p.tile([36, 2048], BF16, tag="t2b", name=f"t2b{b}")
                    for b in range(nblk)]

            # --- ALL DMA dispatched from sync (pure dispatcher) in
            # consumption order, with per-block qk01 slices so no big
            # constant blob delays a block's stream data.  gpsimd's SW
            # queue carries tx2 + blocks 4-7's txn first halves + late
            # constants.  Compute engines (scalar/vector) never touch a
            # DMA queue, so their FIFOs can't block behind a dispatch. ---
            nc.sync.dma_start(out=qk01[:, 0:128], in_=qk01_d[:, 0:128])
            nc.sync.dma_start(out=qk01[:, sh:sh + 128],
                              in_=qk01_d[:, sh:sh + 128])
            for b in range(nblk):
                nc.sync.dma_start(out=txts[b], in_=txt_d[b, :, :])
                nc.sync.dma_start(out=txns[b], in_=txn_d[b, :, :])
                if b == 0:
                    # block 1's qk01 slices first, then the rest
                    nc.sync.dma_start(out=qk01[:, 128:256],
                                      in_=qk01_d[:, 128:256])
                    nc.sync.dma_start(out=qk01[:, sh + 128:sh + 256],
                                      in_=qk01_d[:, sh + 128:sh + 256])
                if b == 1:
                    nc.sync.dma_start(out=qk01[:, 256:sh],
                                      in_=qk01_d[:, 256:sh])
                    nc.sync.dma_start(out=qk01[:, sh + 256:2 * sh],
                                      in_=qk01_d[:, sh + 256:2 * sh])
            # late constants ride after the whole stream (needed ~end)
            nc.sync.dma_start(out=wfxa, in_=wfxa_d[:, :, :])
            nc.sync.dma_start(out=wfxb, in_=wfxb_d[:, :, :])
            nc.sync.dma_start(out=obias, in_=obias_d[:, :])
            nc.sync.dma_start(out=rmask, in_=rmask_d[:, :])
            nc.gpsimd.dma_start(out=qk2, in_=qk2_d[:, :])
            for b in range(nblk):
                nc.gpsimd.dma_start(out=t2bs[b], in_=tx2_d[b, :, :])

            # ---------------- main loop (1-block software pipeline) ----
            # stage A (block b): DMAs + logits + exp  (PE: 12 logits MMs)
            # stage B (block b): transposes + G^T + gsb + tr + gall
            # Emitting A(b+1) before B(b) keeps the PE FIFO free of
            # exp/copy stalls: while scalar runs exp(b+1), the PE chews
            # B(b)'s transposes and G matmuls.
            stateA = {}

            def stage_a(blk):
                txt = txts[blk]
                t2b = t2bs[blk]
                # logits step-outer so the 4 col-strips stream CONCURRENTLY
                l_ps = ps_l.tile([128, 512], F32, name="l_ps")
                for step in range(3):
                    for g4 in range(4):
                        g = blk * 4 + g4
                        osl = slice(g4 * 32, (g4 + 1) * 32)
                        if step == 0:
                            st = qk01[:, g * 32:(g + 1) * 32]
                            mv = txt[:, g4 * 512:(g4 + 1) * 512]
                        elif step == 1:
                            st = qk01[:, sc * H + g * 32:sc * H + (g + 1) * 32]
                            mv = txt[:, 2048 + g4 * 512:2048 + (g4 + 1) * 512]
                        else:
                            st = qk2[:, g * 32:(g + 1) * 32]
                            mv = t2b[:, g4 * 512:(g4 + 1) * 512]
                        nc.tensor.matmul(
                            l_ps[osl, :], st, mv, start=(step == 0),
                            stop=(step == 2), tile_position=(0, g4 * 32))

                # exp in 2 halves (no max-subtraction; -1e30 slots -> 0)
                den0 = smallp.tile([128, 1], F32, tag="den0", name="den0")
                den1 = smallp.tile([128, 1], F32, tag="den1", name="den1")
                attn_e = attnp.tile([128, 512], BF16, tag="ae", name="attn_e")
                nc.scalar.activation(attn_e[:, 0:256], l_ps[:, 0:256],
                                     ACTF.Exp, scale=1.0, accum_out=den0)
                nc.scalar.activation(attn_e[:, 256:512], l_ps[:, 256:512],
                                     ACTF.Exp, scale=1.0, accum_out=den1)
                den = smallp.tile([128, 1], F32, tag="den", name="den")
                nc.vector.tensor_tensor(den, den0, den1, op=ALU.add)
                rden = smallp.tile([128, 1], F32, tag="rden", name="rden")
                nc.vector.reciprocal(rden, den)
                stateA[blk] = (attn_e, rden)

            def stage_b(blk):
                attn_e, rden = stateA.pop(blk)
                txn = txns[blk]
                # 4 window transposes -> attT tiles (off-window cols are 0)
                atcs = []
                for w in range(4):
                    atp = ps_g.tile([128, 128], BF16, tag=f"g{w}",
                                    name=f"atp{w}")
                    nc.tensor.matmul(
                        atp, attn_e[:, w * 128:(w + 1) * 128], eyeb,
                        is_transpose=True, start=True, stop=True)
                    atc = attnp.tile([128, 128], BF16, tag=f"atc{w}",
                                     name=f"atc{w}")
                    if w % 2 == 0:
                        nc.vector.tensor_copy(atc, atp)
                    else:
                        nc.scalar.copy(out=atc, in_=atp)
                    atcs.append(atc)

                # G^T: 16 matmuls accumulate into one clean PSUM tile.
                # stationary = attT window-strip (zero off-diagonal),
                # moving = raw tgtx natural (fp8).  j = g4*4 + jj.
                gt = ps_g.tile([128, 288], F32, tag="gt", name="gt")
                for jj in range(4):
                    for g4 in range(4):
                        j = g4 * 4 + jj
                        nc.tensor.matmul(
                            gt[g4 * 32:(g4 + 1) * 32, :],
                            atcs[jj][:, g4 * 32:(g4 + 1) * 32],
                            txn[:, j * DX:(j + 1) * DX],
                            start=(jj == 0), stop=(jj == 3),
                            tile_position=(0, g4 * 32))

                # normalize by 1/den during the single PSUM->SBUF copy
                # (gt partitions are exactly the l_ps row order (g4,jj,h))
                gsb = attnp.tile([128, 288], BF16, tag="gsb", name="gsb")
                nc.vector.tensor_scalar_mul(gsb[:, 0:128], gt[:, 0:128], rden)
                nc.scalar.activation(gsb[:, 128:288], gt[:, 128:288],
                                     ACTF.Copy, scale=rden)

                # G^T -> G (natural) via 3 PE transposes
                tr = ps_t.tile([128, 384], BF16, tag="tr", name="tr")
                nc.tensor.matmul(tr[:, 0:128], gsb[:, 0:128], eyeb,
                                 is_transpose=True, start=True, stop=True)
                nc.tensor.matmul(tr[:, 128:256], gsb[:, 128:256], eyeb,
                                 is_transpose=True, start=True, stop=True)
                nc.tensor.matmul(tr[0:32, 256:384], gsb[:, 256:288], eyeb,
                                 is_transpose=True, start=True, stop=True)

                # tr cols are (j16, h) j-major; scatter into gall[(c,h), s]
                b0 = blk * 16
                for c in range(3):
                    pn = 128 if c < 2 else 32
                    src = tr[0:pn, c * 128:(c + 1) * 128].rearrange(
                        "p (j h) -> p h j", j=16, h=8)
                    nc.vector.tensor_copy(gall[0:pn, c, :, b0:b0 + 16], src)

            # B(b-1) BEFORE A(b): a data-starved A(b) at the queue head
            # would block B(b-1)'s ready work (engine FIFOs pop in order);
            # mid-kernel the DMA pace leaves enough slack to hide B's
            # exp-dependency wait anyway.
            for blk in range(nblk + 1):
                if blk >= 1:
                    stage_b(blk - 1)
                if blk < nblk:
                    stage_a(blk)

            # ---------------- output projection ----------------
            out_ps = ps_t.tile([sc, DOUT], F32, tag="tr", name="out_ps")
            for c in range(2):
                for h in range(8):
                    k = c * 8 + h
                    nc.tensor.matmul(out_ps, gall[:, c, h, :], wfxa[:, k, :],
                                     start=(k == 0), stop=False)
            for h in range(8):
                nc.tensor.matmul(out_ps, gall[0:32, 2, h, :], wfxb[:, h, :],
                                 start=False, stop=(h == 7))

            # out = out_ps * rmask + obias_m   (obias pre-masked on host)
            out_sb = cp.tile([sc, DOUT], F32, name="out_sb")
            nc.vector.scalar_tensor_tensor(
                out=out_sb, in0=out_ps, scalar=rmask, in1=obias,
                op0=ALU.mult, op1=ALU.add)
            nc.sync.dma_start(out=out_d[:, :], in_=out_sb)

    nc.finalize()
    return nc


def host_prep(src, tgt, rpe, tgt_padding_mask, in_proj_weight, in_proj_bias,
              out_proj_weight, out_proj_bias, rpe_weight, rpe_bias):
    """Host-side folding + layout prep.  Returns per-core input maps."""
    f = np.float32
    scale = f(1.0 / np.sqrt(DH))

    src_f = np.asarray(src, f).reshape(BS, D)
    ipw = np.asarray(in_proj_weight, f)
    ipb = np.asarray(in_proj_bias, f)
    opw = np.asarray(out_proj_weight, f)
    opb = np.asarray(out_proj_bias, f)
    rw = np.asarray(rpe_weight, f)
    rb = np.asarray(rpe_bias, f)

    # ---- q-path fold (host): qk[(f|rpe|sel), s, h] ----
    q_s = (src_f @ ipw[:D].T + ipb[:D]) * scale          # [BS, D]
    wk = ipw[D:2 * D]                                    # [e, d]
    rwk = rw[:D]                                         # [e, r]
    qh = q_s.reshape(BS, H, DH)
    qw = np.einsum('shk,hkf->shf', qh, wk.reshape(H, DH, D))     # [BS,H,D]
    qrw = np.einsum('shk,hkf->shf', qh, rwk.reshape(H, DH, DR))  # [BS,H,DR]
    sel = (np.arange(4)[:, None] == (np.arange(SC) % 4)[None, :]).astype(f)
    qwT = qw.transpose(2, 0, 1).reshape(D, NCORES, SC * H)    # [D, c, s*h]
    qrwT = qrw.transpose(2, 0, 1).reshape(DR, NCORES, SC * H)
    qk01 = np.empty((NCORES, 128, 2 * SC * H), NPBF16)
    qk01[:, :, 0:SC * H] = qwT[0:128].transpose(1, 0, 2).astype(NPBF16)
    qk01[:, :, SC * H:] = qwT[128:256].transpose(1, 0, 2).astype(NPBF16)
    qk2 = np.empty((NCORES, 36, SC * H), NPBF16)
    qk2[:, 0:32] = qrwT.transpose(1, 0, 2).astype(NPBF16)
    selh = np.broadcast_to(sel[:, :, None], (4, SC, H)).reshape(4, SC * H)
    qk2[:, 32:36] = selh.astype(NPBF16)[None]

    # ---- tgtx in both layouts ----
    tgtx = np.concatenate(
        [np.asarray(tgt, f).reshape(BS, T, D),
         np.asarray(rpe, f).reshape(BS, T, DR)], axis=-1)   # [BS, T, DX]
    tgtx16 = tgtx.astype(NPBF16)
    # natural: txn[.., t, j*288+f] (fp8 e3m4 for the G path)
    txn = np.ascontiguousarray(tgtx.reshape(
        NCORES, NBLK, 16, T, DX).transpose(0, 1, 3, 2, 4).reshape(
        NCORES, NBLK, T, 16 * DX).astype(NPF8E3))
    # transposed: [c, blk, f, (g4, jj, t)]
    txtT = tgtx16.reshape(NCORES, NBLK, 4, 4, T, DX).transpose(
        0, 1, 5, 2, 3, 4).reshape(NCORES, NBLK, DX, 2048)
    txt = np.empty((NCORES, NBLK, 128, 4096), NPBF16)
    txt[:, :, :, 0:2048] = txtT[:, :, 0:128]
    txt[:, :, :, 2048:4096] = txtT[:, :, 128:256]
    tx2 = np.empty((NCORES, NBLK, 36, 2048), NPBF16)
    tx2[:, :, 0:32] = txtT[:, :, 256:288]

    # ---- mask rows: M[m, (g4, j, t)] = maskadd if j==m else -1e30 ----
    mask = np.asarray(tgt_padding_mask, bool).reshape(BS, T)
    no_valid = mask.all(-1)
    maskadd = np.where(mask & ~no_valid[:, None], f(-1e30), f(0.0))
    Mfull = np.full((BS, 4, T), -1e30, f).reshape(NCORES, NBLK, 4, 4, 4, T)
    ma_g = maskadd.reshape(NCORES, NBLK, 4, 4, T)
    for m in range(4):
        Mfull[:, :, :, m, m, :] = ma_g[:, :, :, m, :]
    # Mfull dims: [c, blk, g4, m, j, t] -> [c, blk, m, (g4, j, t)]
    tx2[:, :, 32:36] = Mfull.transpose(0, 1, 3, 2, 4, 5).reshape(
        NCORES, NBLK, 4, 2048).astype(NPBF16)

    # ---- output-side folds ----
    wvx = np.concatenate([ipw[2 * D:3 * D], rw[D:2 * D]], axis=1)  # [e, DX]
    wfxa = np.empty((128, 16, DOUT), f)
    wfxb = np.empty((32, 8, DOUT), f)
    for h in range(H):
        hs = slice(h * DH, (h + 1) * DH)
        wfxh = (opw[:, hs] @ wvx[hs, :]).T        # [DX, DOUT]
        wfxa[:, h, :] = wfxh[0:128]
        wfxa[:, 8 + h, :] = wfxh[128:256]
        wfxb[:, h, :] = wfxh[256:288]
    wfxa16 = np.ascontiguousarray(wfxa.astype(NPBF16))
    wfxb16 = np.ascontiguousarray(wfxb.astype(NPBF16))

    obias_row = (opb + opw @ (ipb[2 * D:3 * D] + rb[D:2 * D]))[None, :]
    rowmask = (~no_valid).astype(f)[:, None]                    # [BS, 1]
    obias_m = (obias_row * rowmask).astype(f)                   # [BS, DOUT]

    in_maps = []
    for c in range(NCORES):
        sl = slice(c * SC, (c + 1) * SC)
        in_maps.append({
            "txt": np.ascontiguousarray(txt[c]),
            "txn": np.ascontiguousarray(txn[c]),
            "tx2": np.ascontiguousarray(tx2[c]),
            "qk01": np.ascontiguousarray(qk01[c]),
            "qk2": np.ascontiguousarray(qk2[c]),
            "wfxa": wfxa16,
            "wfxb": wfxb16,
            "obias": np.ascontiguousarray(obias_m[sl]),
            "rmask": np.ascontiguousarray(rowmask[sl]),
        })
    return in_maps


_NC_CACHE = {}


def get_nc(sc=SC):
    if sc not in _NC_CACHE:
        _NC_CACHE[sc] = build(sc)
    return _NC_CACHE[sc]


def run(in_maps, trace=False):
    nc = get_nc(SC)
    return run_bass_kernel_spmd(nc, in_maps, list(range(NCORES)), trace=trace)


def kernel(**inputs):
    in_maps = host_prep(**inputs)
    res = run(in_maps).results
    out = np.concatenate([res[c]["out"] for c in range(NCORES)], axis=0)
    return np.ascontiguousarray(out.reshape(B, S, D))


# revision 45
# speedup vs baseline: 1.0908x; 1.0908x over previous
"""AttentionRPE kernel for 8 Trainium2 NeuronCores — v2 G^T design.

Math (per (b,s) row, T=128 targets, D=256, H=8 heads, DH=32, DR=32):
  q   = src @ Wsrc.T + bsrc                       [D]
  K'  = tgt @ Wk.T + rpe @ Rwk.T                  [T, D]
  att = softmax_h(q_h . K'_h / sqrt(DH))          [H, T]   (masked)
  out = (att @ V')_heads @ Wout.T + bout          [D]

Device formulation (per core, 128 rows, 8 blocks of 16 rows):
  * q-path folded ON HOST into per-row vectors qk[f, (s,h)] (f = 288
    tgt|rpe features).  logits[(s,h), t] = sum_f qk[f,(s,h)] tgtxT[f,t],
    computed 4-row-group col-tiled with the padding mask + off-window
    -1e30 folded in as 4 extra stationary rows (one-hot selector).
  * softmax WITHOUT max-subtraction (logits are O(10), exp is fp32-safe;
    -1e30 slots underflow to exactly 0).  exp -> bf16, den accumulated.
  * 4 PE window-transposes of exp give attT tiles whose off-diagonal
    (wrong-window) columns are EXACT zeros -> the 16 G^T matmuls
    (stationary attT strip, moving raw tgtx natural fp8) accumulate into
    ONE clean PSUM tile G^T[(j,h), f] with no garbage.
  * 1/den (softmax norm) is applied per-partition during the single
    G^T PSUM->SBUF copy (partitions ARE (j,h) there).
  * 3 PE transposes give G[f, (j,h)] -> gall; final out = sum_k
    gall-chunk^T @ wfx-chunk with host-folded wfx = (Wout_h @ Wvx_h).T.

Sharding: 1024 (b,s) rows split contiguously over 8 cores (128 each).
"""

import numpy as np
import ml_dtypes

import concourse.bass as bass
import concourse.bacc as bacc
import concourse.mybir as mybir
from concourse.tile import TileContext
from concourse.masks import make_identity
from concourse.bass_utils import run_bass_kernel_spmd

B, S, T, D = 2, 512, 128, 256
H, DH, DR = 8, 32, 32
DX = D + DR          # 288 = tgt|rpe feature dim
DOUT = D
NCORES = 8
BS = B * S           # 1024 total rows
SC = BS // NCORES    # 128 rows per core
NBLK = SC // 16      # 8 blocks of 16 rows

F32 = mybir.dt.float32
BF16 = mybir.dt.bfloat16
F8E3 = mybir.dt.float8e3
NPBF16 = np.dtype(ml_dtypes.bfloat16)
NPF8E3 = np.dtype(ml_dtypes.float8_e3m4)

AX = mybir.AxisListType
ALU = mybir.AluOpType
ACTF = mybir.ActivationFunctionType


def build(sc=SC):
    assert sc % 16 == 0
    nblk = sc // 16
    nc = bacc.Bacc()

    txt_d = nc.dram_tensor("txt", [nblk, 128, 4096], BF16, kind="ExternalInput")
    txn_d = nc.dram_tensor("txn", [nblk, 128, 4608], F8E3, kind="ExternalInput")
    tx2_d = nc.dram_tensor("tx2", [nblk, 36, 2048], BF16, kind="ExternalInput")
    qk01_d = nc.dram_tensor("qk01", [128, 2 * sc * H], BF16, kind="ExternalInput")
    qk2_d = nc.dram_tensor("qk2", [36, sc * H], BF16, kind="ExternalInput")
    wfxa_d = nc.dram_tensor("wfxa", [128, 16, DOUT], BF16, kind="ExternalInput")
    wfxb_d = nc.dram_tensor("wfxb", [32, 8, DOUT], BF16, kind="ExternalInput")
    obias_d = nc.dram_tensor("obias", [sc, DOUT], F32, kind="ExternalInput")
    rmask_d = nc.dram_tensor("rmask", [sc, 1], F32, kind="ExternalInput")
    out_d = nc.dram_tensor("out", [sc, DOUT], F32, kind="ExternalOutput")

    with TileContext(nc) as tc:
        with (
            tc.tile_pool(name="const", bufs=1) as cp,
            tc.tile_pool(name="txtp", bufs=8) as txtp,
            tc.tile_pool(name="txnp", bufs=8) as txnp,
            tc.tile_pool(name="tx2p", bufs=8) as tx2p,
            tc.tile_pool(name="attnp", bufs=2) as attnp,
            tc.tile_pool(name="smallp", bufs=2) as smallp,
            tc.tile_pool(name="ps_l", bufs=2, space="PSUM") as ps_l,
            tc.tile_pool(name="ps_g", bufs=1, space="PSUM") as ps_g,
            tc.tile_pool(name="ps_t", bufs=1, space="PSUM") as ps_t,
        ):
            # ---------------- constants ----------------
            qk01 = cp.tile([128, 2 * sc * H], BF16, name="qk01")
            sh = sc * H
            qk2 = cp.tile([36, sc * H], BF16, name="qk2")
            eyeb = cp.tile([128, 128], BF16, name="eyeb")
            make_identity(nc, eyeb)
            obias = cp.tile([sc, DOUT], F32, name="obias")
            rmask = cp.tile([sc, 1], F32, name="rmask")
            wfxa = cp.tile([128, 16, DOUT], BF16, name="wfxa")
            wfxb = cp.tile([32, 8, DOUT], BF16, name="wfxb")
            gall = cp.tile([128, 3, 8, sc], BF16, name="gall")

            # --- all stream tiles pre-allocated (bufs = nblk) ---
            txts = [txtp.tile([128, 4096], BF16, tag="txt", name=f"txt{b}")
                    for b in range(nblk)]
            txns = [txnp.tile([128, 4608], F8E3, tag="txn", name=f"txn{b}")
                    for b in range(nblk)]
            t2bs = [tx2p.tile([36, 2048], BF16, tag="t2b", name=f"t2b{b}")
                    for b in range(nblk)]

            # --- ALL DMA dispatched from sync (pure dispatcher) in
            # consumption order, with per-block qk01 slices so no big
            # constant blob delays a block's stream data.  gpsimd's SW
            # queue carries tx2 + blocks 4-7's txn first halves + late
            # constants.  Compute engines (scalar/vector) never touch a
            # DMA queue, so their FIFOs can't block behind a dispatch. ---
            nc.sync.dma_start(out=qk01[:, 0:128], in_=qk01_d[:, 0:128])
            nc.sync.dma_start(out=qk01[:, sh:sh + 128],
                              in_=qk01_d[:, sh:sh + 128])
            for b in range(nblk):
                nc.sync.dma_start(out=txts[b], in_=txt_d[b, :, :])
                nc.sync.dma_start(out=txns[b], in_=txn_d[b, :, :])
                if b == 0:
                    # block 1's qk01 slices first, then the rest
                    nc.sync.dma_start(out=qk01[:, 128:256],
                                      in_=qk01_d[:, 128:256])
                    nc.sync.dma_start(out=qk01[:, sh + 128:sh + 256],
                                      in_=qk01_d[:, sh + 128:sh + 256])
                if b == 1:
                    nc.sync.dma_start(out=qk01[:, 256:sh],
                                      in_=qk01_d[:, 256:sh])
                    nc.sync.dma_start(out=qk01[:, sh + 256:2 * sh],
                                      in_=qk01_d[:, sh + 256:2 * sh])
            # late constants ride after the whole stream (needed ~end)
            nc.sync.dma_start(out=wfxa, in_=wfxa_d[:, :, :])
            nc.sync.dma_start(out=wfxb, in_=wfxb_d[:, :, :])
            nc.sync.dma_start(out=obias, in_=obias_d[:, :])
            nc.sync.dma_start(out=rmask, in_=rmask_d[:, :])
            nc.gpsimd.dma_start(out=qk2, in_=qk2_d[:, :])
            for b in range(nblk):
                nc.gpsimd.dma_start(out=t2bs[b], in_=tx2_d[b, :, :])

            # ---------------- main loop (1-block software pipeline) ----
            # stage A (block b): DMAs + logits + exp  (PE: 12 logits MMs)
            # stage B (block b): transposes + G^T + gsb + tr + gall
            # Emitting A(b+1) before B(b) keeps the PE FIFO free of
            # exp/copy stalls: while scalar runs exp(b+1), the PE chews
            # B(b)'s transposes and G matmuls.
            stateA = {}

            def stage_a(blk):
                txt = txts[blk]
                t2b = t2bs[blk]
                # logits step-outer so the 4 col-strips stream CONCURRENTLY
                l_ps = ps_l.tile([128, 512], F32, name="l_ps")
                for step in range(3):
                    for g4 in range(4):
                        g = blk * 4 + g4
                        osl = slice(g4 * 32, (g4 + 1) * 32)
                        if step == 0:
                            st = qk01[:, g * 32:(g + 1) * 32]
                            mv = txt[:, g4 * 512:(g4 + 1) * 512]
                        elif step == 1:
                            st = qk01[:, sc * H + g * 32:sc * H + (g + 1) * 32]
                            mv = txt[:, 2048 + g4 * 512:2048 + (g4 + 1) * 512]
                        else:
                            st = qk2[:, g * 32:(g + 1) * 32]
                            mv = t2b[:, g4 * 512:(g4 + 1) * 512]
                        nc.tensor.matmul(
                            l_ps[osl, :], st, mv, start=(step == 0),
                            stop=(step == 2), tile_position=(0, g4 * 32))

                # exp in 2 halves (no max-subtraction; -1e30 slots -> 0)
                den0 = smallp.tile([128, 1], F32, tag="den0", name="den0")
                den1 = smallp.tile([128, 1], F32, tag="den1", name="den1")
                attn_e = attnp.tile([128, 512], BF16, tag="ae", name="attn_e")
                nc.scalar.activation(attn_e[:, 0:256], l_ps[:, 0:256],
                                     ACTF.Exp, scale=1.0, accum_out=den0)
                nc.scalar.activation(attn_e[:, 256:512], l_ps[:, 256:512],
                                     ACTF.Exp, scale=1.0, accum_out=den1)
                den = smallp.tile([128, 1], F32, tag="den", name="den")
                nc.vector.tensor_tensor(den, den0, den1, op=ALU.add)
                rden = smallp.tile([128, 1], F32, tag="rden", name="rden")
                nc.vector.reciprocal(rden, den)
                stateA[blk] = (attn_e, rden)

            def stage_b(blk):
                attn_e, rden = stateA.pop(blk)
                txn = txns[blk]
                # 4 window transposes -> attT tiles (off-window cols are 0)
                atcs = []
                for w in range(4):
                    atp = ps_g.tile([128, 128], BF16, tag=f"g{w}",
                                    name=f"atp{w}")
                    nc.tensor.matmul(
                        atp, attn_e[:, w * 128:(w + 1) * 128], eyeb,
                        is_transpose=True, start=True, stop=True)
                    atc = attnp.tile([128, 128], BF16, tag=f"atc{w}",
                                     name=f"atc{w}")
                    if w % 2 == 0:
                        nc.vector.tensor_copy(atc, atp)
                    else:
                        nc.scalar.copy(out=atc, in_=atp)
                    atcs.append(atc)

                # G^T: 16 matmuls accumulate into one clean PSUM tile.
                # stationary = attT window-strip (zero off-diagonal),
                # moving = raw tgtx natural (fp8).  j = g4*4 + jj.
                gt = ps_g.tile([128, 288], F32, tag="gt", name="gt")
                for jj in range(4):
                    for g4 in range(4):
                        j = g4 * 4 + jj
                        nc.tensor.matmul(
                            gt[g4 * 32:(g4 + 1) * 32, :],
                            atcs[jj][:, g4 * 32:(g4 + 1) * 32],
                            txn[:, j * DX:(j + 1) * DX],
                            start=(jj == 0), stop=(jj == 3),
                            tile_position=(0, g4 * 32))

                # normalize by 1/den during the single PSUM->SBUF copy
                # (gt partitions are exactly the l_ps row order (g4,jj,h))
                gsb = attnp.tile([128, 288], BF16, tag="gsb", name="gsb")
                nc.vector.tensor_scalar_mul(gsb[:, 0:128], gt[:, 0:128], rden)
                nc.scalar.activation(gsb[:, 128:288], gt[:, 128:288],
                                     ACTF.Copy, scale=rden)

                # G^T -> G (natural) via 3 PE transposes
                tr = ps_t.tile([128, 384], BF16, tag="tr", name="tr")
                nc.tensor.matmul(tr[:, 0:128], gsb[:, 0:128], eyeb,
                                 is_transpose=True, start=True, stop=True)
                nc.tensor.matmul(tr[:, 128:256], gsb[:, 128:256], eyeb,
                                 is_transpose=True, start=True, stop=True)
                nc.tensor.matmul(tr[0:32, 256:384], gsb[:, 256:288], eyeb,
                                 is_transpose=True, start=True, stop=True)

                # tr cols are (j16, h) j-major; scatter into gall[(c,h), s]
                b0 = blk * 16
                for c in range(3):
                    pn = 128 if c < 2 else 32
                    src = tr[0:pn, c * 128:(c + 1) * 128].rearrange(
                        "p (j h) -> p h j", j=16, h=8)
                    nc.vector.tensor_copy(gall[0:pn, c, :, b0:b0 + 16], src)

            for blk in range(nblk + 1):
                if blk < nblk:
                    stage_a(blk)
                if blk >= 1:
                    stage_b(blk - 1)

            # ---------------- output projection ----------------
            out_ps = ps_t.tile([sc, DOUT], F32, tag="tr", name="out_ps")
            for c in range(2):
                for h in range(8):
                    k = c * 8 + h
                    nc.tensor.matmul(out_ps, gall[:, c, h, :], wfxa[:, k, :],
                                     start=(k == 0), stop=False)
            for h in range(8):
                nc.tensor.matmul(out_ps, gall[0:32, 2, h, :], wfxb[:, h, :],
                                 start=False, stop=(h == 7))

            # out = out_ps * rmask + obias_m   (obias pre-masked on host)
            out_sb = cp.tile([sc, DOUT], F32, name="out_sb")
            nc.vector.scalar_tensor_tensor(
                out=out_sb, in0=out_ps, scalar=rmask, in1=obias,
                op0=ALU.mult, op1=ALU.add)
            nc.sync.dma_start(out=out_d[:, :], in_=out_sb)

    nc.finalize()
    return nc


def host_prep(src, tgt, rpe, tgt_padding_mask, in_proj_weight, in_proj_bias,
              out_proj_weight, out_proj_bias, rpe_weight, rpe_bias):
    """Host-side folding + layout prep.  Returns per-core input maps."""
    f = np.float32
    scale = f(1.0 / np.sqrt(DH))

    src_f = np.asarray(src, f).reshape(BS, D)
    ipw = np.asarray(in_proj_weight, f)
    ipb = np.asarray(in_proj_bias, f)
    opw = np.asarray(out_proj_weight, f)
    opb = np.asarray(out_proj_bias, f)
    rw = np.asarray(rpe_weight, f)
    rb = np.asarray(rpe_bias, f)

    # ---- q-path fold (host): qk[(f|rpe|sel), s, h] ----
    q_s = (src_f @ ipw[:D].T + ipb[:D]) * scale          # [BS, D]
    wk = ipw[D:2 * D]                                    # [e, d]
    rwk = rw[:D]                                         # [e, r]
    qh = q_s.reshape(BS, H, DH)
    qw = np.einsum('shk,hkf->shf', qh, wk.reshape(H, DH, D))     # [BS,H,D]
    qrw = np.einsum('shk,hkf->shf', qh, rwk.reshape(H, DH, DR))  # [BS,H,DR]
    sel = (np.arange(4)[:, None] == (np.arange(SC) % 4)[None, :]).astype(f)
    qwT = qw.transpose(2, 0, 1).reshape(D, NCORES, SC * H)    # [D, c, s*h]
    qrwT = qrw.transpose(2, 0, 1).reshape(DR, NCORES, SC * H)
    qk01 = np.empty((NCORES, 128, 2 * SC * H), NPBF16)
    qk01[:, :, 0:SC * H] = qwT[0:128].transpose(1, 0, 2).astype(NPBF16)
    qk01[:, :, SC * H:] = qwT[128:256].transpose(1, 0, 2).astype(NPBF16)
    qk2 = np.empty((NCORES, 36, SC * H), NPBF16)
    qk2[:, 0:32] = qrwT.transpose(1, 0, 2).astype(NPBF16)
    selh = np.broadcast_to(sel[:, :, None], (4, SC, H)).reshape(4, SC * H)
    qk2[:, 32:36] = selh.astype(NPBF16)[None]

    # ---- tgtx in both layouts ----
    tgtx = np.concatenate(
        [np.asarray(tgt, f).reshape(BS, T, D),
         np.asarray(rpe, f).reshape(BS, T, DR)], axis=-1)   # [BS, T, DX]
    tgtx16 = tgtx.astype(NPBF16)
    # natural: txn[.., t, j*288+f] (fp8 e3m4 for the G path)
    txn = np.ascontiguousarray(tgtx.reshape(
        NCORES, NBLK, 16, T, DX).transpose(0, 1, 3, 2, 4).reshape(
        NCORES, NBLK, T, 16 * DX).astype(NPF8E3))
    # transposed: [c, blk, f, (g4, jj, t)]
    txtT = tgtx16.reshape(NCORES, NBLK, 4, 4, T, DX).transpose(
        0, 1, 5, 2, 3, 4).reshape(NCORES, NBLK, DX, 2048)
    txt = np.empty((NCORES, NBLK, 128, 4096), NPBF16)
    txt[:, :, :, 0:2048] = txtT[:, :, 0:128]
    txt[:, :, :, 2048:4096] = txtT[:, :, 128:256]
    tx2 = np.empty((NCORES, NBLK, 36, 2048), NPBF16)
    tx2[:, :, 0:32] = txtT[:, :, 256:288]

    # ---- mask rows: M[m, (g4, j, t)] = maskadd if j==m else -1e30 ----
    mask = np.asarray(tgt_padding_mask, bool).reshape(BS, T)
    no_valid = mask.all(-1)
    maskadd = np.where(mask & ~no_valid[:, None], f(-1e30), f(0.0))
    Mfull = np.full((BS, 4, T), -1e30, f).reshape(NCORES, NBLK, 4, 4, 4, T)
    ma_g = maskadd.reshape(NCORES, NBLK, 4, 4, T)
    for m in range(4):
        Mfull[:, :, :, m, m, :] = ma_g[:, :, :, m, :]
    # Mfull dims: [c, blk, g4, m, j, t] -> [c, blk, m, (g4, j, t)]
    tx2[:, :, 32:36] = Mfull.transpose(0, 1, 3, 2, 4, 5).reshape(
        NCORES, NBLK, 4, 2048).astype(NPBF16)

    # ---- output-side folds ----
    wvx = np.concatenate([ipw[2 * D:3 * D], rw[D:2 * D]], axis=1)  # [e, DX]
    wfxa = np.empty((128, 16, DOUT), f)
    wfxb = np.empty((32, 8, DOUT), f)
    for h in range(H):
        hs = slice(h * DH, (h + 1) * DH)
        wfxh = (opw[:, hs] @ wvx[hs, :]).T        # [DX, DOUT]
        wfxa[:, h, :] = wfxh[0:128]
        wfxa[:, 8 + h, :] = wfxh[128:256]
        wfxb[:, h, :] = wfxh[256:288]
    wfxa16 = np.ascontiguousarray(wfxa.astype(NPBF16))
    wfxb16 = np.ascontiguousarray(wfxb.astype(NPBF16))

    obias_row = (opb + opw @ (ipb[2 * D:3 * D] + rb[D:2 * D]))[None, :]
    rowmask = (~no_valid).astype(f)[:, None]                    # [BS, 1]
    obias_m = (obias_row * rowmask).astype(f)                   # [BS, DOUT]

    in_maps = []
    for c in range(NCORES):
        sl = slice(c * SC, (c + 1) * SC)
        in_maps.append({
            "txt": np.ascontiguousarray(txt[c]),
            "txn": np.ascontiguousarray(txn[c]),
            "tx2": np.ascontiguousarray(tx2[c]),
            "qk01": np.ascontiguousarray(qk01[c]),
            "qk2": np.ascontiguousarray(qk2[c]),
            "wfxa": wfxa16,
            "wfxb": wfxb16,
            "obias": np.ascontiguousarray(obias_m[sl]),
            "rmask": np.ascontiguousarray(rowmask[sl]),
        })
    return in_maps


_NC_CACHE = {}


def get_nc(sc=SC):
    if sc not in _NC_CACHE:
        _NC_CACHE[sc] = build(sc)
    return _NC_CACHE[sc]


def run(in_maps, trace=False):
    nc = get_nc(SC)
    return run_bass_kernel_spmd(nc, in_maps, list(range(NCORES)), trace=trace)


def kernel(**inputs):
    in_maps = host_prep(**inputs)
    res = run(in_maps).results
    out = np.concatenate([res[c]["out"] for c in range(NCORES)], axis=0)
    return np.ascontiguousarray(out.reshape(B, S, D))


# revision 48
# speedup vs baseline: 1.0999x; 1.0083x over previous
"""AttentionRPE kernel for 8 Trainium2 NeuronCores — v2 G^T design.

Math (per (b,s) row, T=128 targets, D=256, H=8 heads, DH=32, DR=32):
  q   = src @ Wsrc.T + bsrc                       [D]
  K'  = tgt @ Wk.T + rpe @ Rwk.T                  [T, D]
  att = softmax_h(q_h . K'_h / sqrt(DH))          [H, T]   (masked)
  out = (att @ V')_heads @ Wout.T + bout          [D]

Device formulation (per core, 128 rows, 8 blocks of 16 rows):
  * q-path folded ON HOST into per-row vectors qk[f, (s,h)] (f = 288
    tgt|rpe features).  logits[(s,h), t] = sum_f qk[f,(s,h)] tgtxT[f,t],
    computed 4-row-group col-tiled with the padding mask + off-window
    -1e30 folded in as 4 extra stationary rows (one-hot selector).
  * softmax WITHOUT max-subtraction (logits are O(10), exp is fp32-safe;
    -1e30 slots underflow to exactly 0).  exp -> bf16, den accumulated.
  * 4 PE window-transposes of exp give attT tiles whose off-diagonal
    (wrong-window) columns are EXACT zeros -> the 16 G^T matmuls
    (stationary attT strip, moving raw tgtx natural fp8) accumulate into
    ONE clean PSUM tile G^T[(j,h), f] with no garbage.
  * 1/den (softmax norm) is applied per-partition during the single
    G^T PSUM->SBUF copy (partitions ARE (j,h) there).
  * 3 PE transposes give G[f, (j,h)] -> gall; final out = sum_k
    gall-chunk^T @ wfx-chunk with host-folded wfx = (Wout_h @ Wvx_h).T.

Sharding: 1024 (b,s) rows split contiguously over 8 cores (128 each).
"""

import numpy as np
import ml_dtypes

import concourse.bass as bass
import concourse.bacc as bacc
import concourse.mybir as mybir
from concourse.tile import TileContext
from concourse.masks import make_identity
from concourse.bass_utils import run_bass_kernel_spmd

B, S, T, D = 2, 512, 128, 256
H, DH, DR = 8, 32, 32
DX = D + DR          # 288 = tgt|rpe feature dim
DOUT = D
NCORES = 8
BS = B * S           # 1024 total rows
SC = BS // NCORES    # 128 rows per core
NBLK = SC // 16      # 8 blocks of 16 rows

F32 = mybir.dt.float32
BF16 = mybir.dt.bfloat16
F8E3 = mybir.dt.float8e3
NPBF16 = np.dtype(ml_dtypes.bfloat16)
NPF8E3 = np.dtype(ml_dtypes.float8_e3m4)

AX = mybir.AxisListType
ALU = mybir.AluOpType
ACTF = mybir.ActivationFunctionType


def build(sc=SC):
    assert sc % 16 == 0
    nblk = sc // 16
    nc = bacc.Bacc()

    txt_d = nc.dram_tensor("txt", [nblk, 128, 4096], BF16, kind="ExternalInput")
    txn_d = nc.dram_tensor("txn", [nblk, 128, 4608], F8E3, kind="ExternalInput")
    tx2_d = nc.dram_tensor("tx2", [nblk, 36, 2048], BF16, kind="ExternalInput")
    qk01_d = nc.dram_tensor("qk01", [128, 2 * sc * H], BF16, kind="ExternalInput")
    qk2_d = nc.dram_tensor("qk2", [36, sc * H], BF16, kind="ExternalInput")
    wfxa_d = nc.dram_tensor("wfxa", [128, 16, DOUT], BF16, kind="ExternalInput")
    wfxb_d = nc.dram_tensor("wfxb", [32, 8, DOUT], BF16, kind="ExternalInput")
    obias_d = nc.dram_tensor("obias", [sc, DOUT], F32, kind="ExternalInput")
    rmask_d = nc.dram_tensor("rmask", [sc, 1], F32, kind="ExternalInput")
    out_d = nc.dram_tensor("out", [sc, DOUT], F32, kind="ExternalOutput")

    with TileContext(nc) as tc:
        with (
            tc.tile_pool(name="const", bufs=1) as cp,
            tc.tile_pool(name="txtp", bufs=8) as txtp,
            tc.tile_pool(name="txnp", bufs=8) as txnp,
            tc.tile_pool(name="tx2p", bufs=8) as tx2p,
            tc.tile_pool(name="attnp", bufs=2) as attnp,
            tc.tile_pool(name="smallp", bufs=2) as smallp,
            tc.tile_pool(name="ps_l", bufs=2, space="PSUM") as ps_l,
            tc.tile_pool(name="ps_g", bufs=1, space="PSUM") as ps_g,
            tc.tile_pool(name="ps_t", bufs=1, space="PSUM") as ps_t,
        ):
            # ---------------- constants ----------------
            qk01 = cp.tile([128, 2 * sc * H], BF16, name="qk01")
            sh = sc * H
            qk2 = cp.tile([36, sc * H], BF16, name="qk2")
            eyeb = cp.tile([128, 128], BF16, name="eyeb")
            make_identity(nc, eyeb)
            obias = cp.tile([sc, DOUT], F32, name="obias")
            rmask = cp.tile([sc, 1], F32, name="rmask")
            wfxa = cp.tile([128, 16, DOUT], BF16, name="wfxa")
            wfxb = cp.tile([32, 8, DOUT], BF16, name="wfxb")
            gall = cp.tile([128, 3, 8, sc], BF16, name="gall")

            # --- all stream tiles pre-allocated (bufs = nblk) ---
            txts = [txtp.tile([128, 4096], BF16, tag="txt", name=f"txt{b}")
                    for b in range(nblk)]
            txns = [txnp.tile([128, 4608], F8E3, tag="txn", name=f"txn{b}")
                    for b in range(nblk)]
            t2bs = [tx2p.tile([36, 2048], BF16, tag="t2b", name=f"t2b{b}")
                    for b in range(nblk)]

            # --- ALL DMA dispatched from sync (pure dispatcher) in
            # consumption order, with per-block qk01 slices so no big
            # constant blob delays a block's stream data.  gpsimd's SW
            # queue carries tx2 + blocks 4-7's txn first halves + late
            # constants.  Compute engines (scalar/vector) never touch a
            # DMA queue, so their FIFOs can't block behind a dispatch. ---
            nc.sync.dma_start(out=qk01[:, 0:128], in_=qk01_d[:, 0:128])
            nc.sync.dma_start(out=qk01[:, sh:sh + 128],
                              in_=qk01_d[:, sh:sh + 128])
            for b in range(nblk):
                nc.sync.dma_start(out=txts[b], in_=txt_d[b, :, :])
                nc.sync.dma_start(out=txns[b], in_=txn_d[b, :, :])
                if b == 0:
                    # block 1's qk01 slices first, then the rest
                    nc.sync.dma_start(out=qk01[:, 128:256],
                                      in_=qk01_d[:, 128:256])
                    nc.sync.dma_start(out=qk01[:, sh + 128:sh + 256],
                                      in_=qk01_d[:, sh + 128:sh + 256])
                if b == 1:
                    nc.sync.dma_start(out=qk01[:, 256:sh],
                                      in_=qk01_d[:, 256:sh])
                    nc.sync.dma_start(out=qk01[:, sh + 256:2 * sh],
                                      in_=qk01_d[:, sh + 256:2 * sh])
            # late constants ride after the whole stream (needed ~end)
            nc.sync.dma_start(out=wfxa, in_=wfxa_d[:, :, :])
            nc.sync.dma_start(out=wfxb, in_=wfxb_d[:, :, :])
            nc.sync.dma_start(out=obias, in_=obias_d[:, :])
            nc.sync.dma_start(out=rmask, in_=rmask_d[:, :])
            nc.gpsimd.dma_start(out=qk2, in_=qk2_d[:, :])
            for b in range(nblk):
                nc.gpsimd.dma_start(out=t2bs[b], in_=tx2_d[b, :, :])

            # ---------------- main loop (1-block software pipeline) ----
            # stage A (block b): DMAs + logits + exp  (PE: 12 logits MMs)
            # stage B (block b): transposes + G^T + gsb + tr + gall
            # Emitting A(b+1) before B(b) keeps the PE FIFO free of
            # exp/copy stalls: while scalar runs exp(b+1), the PE chews
            # B(b)'s transposes and G matmuls.
            stateA = {}

            def stage_a(blk):
                txt = txts[blk]
                t2b = t2bs[blk]
                # logits step-outer so the 4 col-strips stream CONCURRENTLY
                l_ps = ps_l.tile([128, 512], F32, name="l_ps")
                for step in range(3):
                    for g4 in range(4):
                        g = blk * 4 + g4
                        osl = slice(g4 * 32, (g4 + 1) * 32)
                        if step == 0:
                            st = qk01[:, g * 32:(g + 1) * 32]
                            mv = txt[:, g4 * 512:(g4 + 1) * 512]
                        elif step == 1:
                            st = qk01[:, sc * H + g * 32:sc * H + (g + 1) * 32]
                            mv = txt[:, 2048 + g4 * 512:2048 + (g4 + 1) * 512]
                        else:
                            st = qk2[:, g * 32:(g + 1) * 32]
                            mv = t2b[:, g4 * 512:(g4 + 1) * 512]
                        nc.tensor.matmul(
                            l_ps[osl, :], st, mv, start=(step == 0),
                            stop=(step == 2), tile_position=(0, g4 * 32))

                # exp in 2 halves (no max-subtraction; -1e30 slots -> 0)
                den0 = smallp.tile([128, 1], F32, tag="den0", name="den0")
                den1 = smallp.tile([128, 1], F32, tag="den1", name="den1")
                attn_e = attnp.tile([128, 512], BF16, tag="ae", name="attn_e")
                nc.scalar.activation(attn_e[:, 0:256], l_ps[:, 0:256],
                                     ACTF.Exp, scale=1.0, accum_out=den0)
                nc.scalar.activation(attn_e[:, 256:512], l_ps[:, 256:512],
                                     ACTF.Exp, scale=1.0, accum_out=den1)
                if blk == nblk - 1:
                    # keep-warm: tiny matmuls bridge the PE lull while the
                    # last blocks wait on DMA, so HAM stays at K=8/8 and
                    # the final projection runs at full clock
                    for i in range(2):
                        wz = ps_g.tile([32, 32], F32, tag=f"g{i}",
                                       name=f"warma{i}")
                        nc.tensor.matmul(wz, eyeb[:, 0:32], eyeb[:, 0:32],
                                         start=True, stop=True)
                den = smallp.tile([128, 1], F32, tag="den", name="den")
                nc.vector.tensor_tensor(den, den0, den1, op=ALU.add)
                rden = smallp.tile([128, 1], F32, tag="rden", name="rden")
                nc.vector.reciprocal(rden, den)
                stateA[blk] = (attn_e, rden)

            def stage_b(blk):
                attn_e, rden = stateA.pop(blk)
                txn = txns[blk]
                # 4 window transposes -> attT tiles (off-window cols are 0)
                atcs = []
                for w in range(4):
                    atp = ps_g.tile([128, 128], BF16, tag=f"g{w}",
                                    name=f"atp{w}")
                    nc.tensor.matmul(
                        atp, attn_e[:, w * 128:(w + 1) * 128], eyeb,
                        is_transpose=True, start=True, stop=True)
                    atc = attnp.tile([128, 128], BF16, tag=f"atc{w}",
                                     name=f"atc{w}")
                    if w % 2 == 0:
                        nc.vector.tensor_copy(atc, atp)
                    else:
                        nc.scalar.copy(out=atc, in_=atp)
                    atcs.append(atc)

                # G^T: 16 matmuls accumulate into one clean PSUM tile.
                # stationary = attT window-strip (zero off-diagonal),
                # moving = raw tgtx natural (fp8).  j = g4*4 + jj.
                gt = ps_g.tile([128, 288], F32, tag="gt", name="gt")
                for jj in range(4):
                    for g4 in range(4):
                        j = g4 * 4 + jj
                        nc.tensor.matmul(
                            gt[g4 * 32:(g4 + 1) * 32, :],
                            atcs[jj][:, g4 * 32:(g4 + 1) * 32],
                            txn[:, j * DX:(j + 1) * DX],
                            start=(jj == 0), stop=(jj == 3),
                            tile_position=(0, g4 * 32))

                if blk >= nblk - 2:
                    for i in range(2):
                        wz = ps_g.tile([32, 32], F32, tag=f"g{i}",
                                       name=f"warmb{i}")
                        nc.tensor.matmul(wz, eyeb[:, 0:32], eyeb[:, 0:32],
                                         start=True, stop=True)

                # normalize by 1/den during the single PSUM->SBUF copy
                # (gt partitions are exactly the l_ps row order (g4,jj,h))
                gsb = attnp.tile([128, 288], BF16, tag="gsb", name="gsb")
                nc.vector.tensor_scalar_mul(gsb[:, 0:128], gt[:, 0:128], rden)
                nc.scalar.activation(gsb[:, 128:288], gt[:, 128:288],
                                     ACTF.Copy, scale=rden)

                # G^T -> G (natural) via 3 PE transposes
                tr = ps_t.tile([128, 384], BF16, tag="tr", name="tr")
                nc.tensor.matmul(tr[:, 0:128], gsb[:, 0:128], eyeb,
                                 is_transpose=True, start=True, stop=True)
                nc.tensor.matmul(tr[:, 128:256], gsb[:, 128:256], eyeb,
                                 is_transpose=True, start=True, stop=True)
                nc.tensor.matmul(tr[0:32, 256:384], gsb[:, 256:288], eyeb,
                                 is_transpose=True, start=True, stop=True)
                if blk >= nblk - 2:
                    for i in range(2, 4):
                        wz = ps_g.tile([32, 32], F32, tag=f"g{i}",
                                       name=f"warmc{i}")
                        nc.tensor.matmul(wz, eyeb[:, 0:32], eyeb[:, 0:32],
                                         start=True, stop=True)

                # tr cols are (j16, h) j-major; scatter into gall[(c,h), s]
                b0 = blk * 16
                for c in range(3):
                    pn = 128 if c < 2 else 32
                    src = tr[0:pn, c * 128:(c + 1) * 128].rearrange(
                        "p (j h) -> p h j", j=16, h=8)
                    nc.vector.tensor_copy(gall[0:pn, c, :, b0:b0 + 16], src)

            for blk in range(nblk + 1):
                if blk < nblk:
                    stage_a(blk)
                if blk >= 1:
                    stage_b(blk - 1)

            # ---------------- output projection ----------------
            out_ps = ps_t.tile([sc, DOUT], F32, tag="tr", name="out_ps")
            for c in range(2):
                for h in range(8):
                    k = c * 8 + h
                    nc.tensor.matmul(out_ps, gall[:, c, h, :], wfxa[:, k, :],
                                     start=(k == 0), stop=False)
            for h in range(8):
                nc.tensor.matmul(out_ps, gall[0:32, 2, h, :], wfxb[:, h, :],
                                 start=False, stop=(h == 7))

            # out = out_ps * rmask + obias_m   (obias pre-masked on host)
            out_sb = cp.tile([sc, DOUT], F32, name="out_sb")
            nc.vector.scalar_tensor_tensor(
                out=out_sb, in0=out_ps, scalar=rmask, in1=obias,
                op0=ALU.mult, op1=ALU.add)
            nc.sync.dma_start(out=out_d[:, :], in_=out_sb)

    nc.finalize()
    return nc


def host_prep(src, tgt, rpe, tgt_padding_mask, in_proj_weight, in_proj_bias,
              out_proj_weight, out_proj_bias, rpe_weight, rpe_bias):
    """Host-side folding + layout prep.  Returns per-core input maps."""
    f = np.float32
    scale = f(1.0 / np.sqrt(DH))

    src_f = np.asarray(src, f).reshape(BS, D)
    ipw = np.asarray(in_proj_weight, f)
    ipb = np.asarray(in_proj_bias, f)
    opw = np.asarray(out_proj_weight, f)
    opb = np.asarray(out_proj_bias, f)
    rw = np.asarray(rpe_weight, f)
    rb = np.asarray(rpe_bias, f)

    # ---- q-path fold (host): qk[(f|rpe|sel), s, h] ----
    q_s = (src_f @ ipw[:D].T + ipb[:D]) * scale          # [BS, D]
    wk = ipw[D:2 * D]                                    # [e, d]
    rwk = rw[:D]                                         # [e, r]
    qh = q_s.reshape(BS, H, DH)
    qw = np.einsum('shk,hkf->shf', qh, wk.reshape(H, DH, D))     # [BS,H,D]
    qrw = np.einsum('shk,hkf->shf', qh, rwk.reshape(H, DH, DR))  # [BS,H,DR]
    sel = (np.arange(4)[:, None] == (np.arange(SC) % 4)[None, :]).astype(f)
    qwT = qw.transpose(2, 0, 1).reshape(D, NCORES, SC * H)    # [D, c, s*h]
    qrwT = qrw.transpose(2, 0, 1).reshape(DR, NCORES, SC * H)
    qk01 = np.empty((NCORES, 128, 2 * SC * H), NPBF16)
    qk01[:, :, 0:SC * H] = qwT[0:128].transpose(1, 0, 2).astype(NPBF16)
    qk01[:, :, SC * H:] = qwT[128:256].transpose(1, 0, 2).astype(NPBF16)
    qk2 = np.empty((NCORES, 36, SC * H), NPBF16)
    qk2[:, 0:32] = qrwT.transpose(1, 0, 2).astype(NPBF16)
    selh = np.broadcast_to(sel[:, :, None], (4, SC, H)).reshape(4, SC * H)
    qk2[:, 32:36] = selh.astype(NPBF16)[None]

    # ---- tgtx in both layouts ----
    tgtx = np.concatenate(
        [np.asarray(tgt, f).reshape(BS, T, D),
         np.asarray(rpe, f).reshape(BS, T, DR)], axis=-1)   # [BS, T, DX]
    tgtx16 = tgtx.astype(NPBF16)
    # natural: txn[.., t, j*288+f] (fp8 e3m4 for the G path)
    txn = np.ascontiguousarray(tgtx.reshape(
        NCORES, NBLK, 16, T, DX).transpose(0, 1, 3, 2, 4).reshape(
        NCORES, NBLK, T, 16 * DX).astype(NPF8E3))
    # transposed: [c, blk, f, (g4, jj, t)]
    txtT = tgtx16.reshape(NCORES, NBLK, 4, 4, T, DX).transpose(
        0, 1, 5, 2, 3, 4).reshape(NCORES, NBLK, DX, 2048)
    txt = np.empty((NCORES, NBLK, 128, 4096), NPBF16)
    txt[:, :, :, 0:2048] = txtT[:, :, 0:128]
    txt[:, :, :, 2048:4096] = txtT[:, :, 128:256]
    tx2 = np.empty((NCORES, NBLK, 36, 2048), NPBF16)
    tx2[:, :, 0:32] = txtT[:, :, 256:288]

    # ---- mask rows: M[m, (g4, j, t)] = maskadd if j==m else -1e30 ----
    mask = np.asarray(tgt_padding_mask, bool).reshape(BS, T)
    no_valid = mask.all(-1)
    maskadd = np.where(mask & ~no_valid[:, None], f(-1e30), f(0.0))
    Mfull = np.full((BS, 4, T), -1e30, f).reshape(NCORES, NBLK, 4, 4, 4, T)
    ma_g = maskadd.reshape(NCORES, NBLK, 4, 4, T)
    for m in range(4):
        Mfull[:, :, :, m, m, :] = ma_g[:, :, :, m, :]
    # Mfull dims: [c, blk, g4, m, j, t] -> [c, blk, m, (g4, j, t)]
    tx2[:, :, 32:36] = Mfull.transpose(0, 1, 3, 2, 4, 5).reshape(
        NCORES, NBLK, 4, 2048).astype(NPBF16)

    # ---- output-side folds ----
    wvx = np.concatenate([ipw[2 * D:3 * D], rw[D:2 * D]], axis=1)  # [e, DX]
    wfxa = np.empty((128, 16, DOUT), f)
    wfxb = np.empty((32, 8, DOUT), f)
    for h in range(H):
        hs = slice(h * DH, (h + 1) * DH)
        wfxh = (opw[:, hs] @ wvx[hs, :]).T        # [DX, DOUT]
        wfxa[:, h, :] = wfxh[0:128]
        wfxa[:, 8 + h, :] = wfxh[128:256]
        wfxb[:, h, :] = wfxh[256:288]
    wfxa16 = np.ascontiguousarray(wfxa.astype(NPBF16))
    wfxb16 = np.ascontiguousarray(wfxb.astype(NPBF16))

    obias_row = (opb + opw @ (ipb[2 * D:3 * D] + rb[D:2 * D]))[None, :]
    rowmask = (~no_valid).astype(f)[:, None]                    # [BS, 1]
    obias_m = (obias_row * rowmask).astype(f)                   # [BS, DOUT]

    in_maps = []
    for c in range(NCORES):
        sl = slice(c * SC, (c + 1) * SC)
        in_maps.append({
            "txt": np.ascontiguousarray(txt[c]),
            "txn": np.ascontiguousarray(txn[c]),
            "tx2": np.ascontiguousarray(tx2[c]),
            "qk01": np.ascontiguousarray(qk01[c]),
            "qk2": np.ascontiguousarray(qk2[c]),
            "wfxa": wfxa16,
            "wfxb": wfxb16,
            "obias": np.ascontiguousarray(obias_m[sl]),
            "rmask": np.ascontiguousarray(rowmask[sl]),
        })
    return in_maps


_NC_CACHE = {}


def get_nc(sc=SC):
    if sc not in _NC_CACHE:
        _NC_CACHE[sc] = build(sc)
    return _NC_CACHE[sc]


def run(in_maps, trace=False):
    nc = get_nc(SC)
    return run_bass_kernel_spmd(nc, in_maps, list(range(NCORES)), trace=trace)


def kernel(**inputs):
    in_maps = host_prep(**inputs)
    res = run(in_maps).results
    out = np.concatenate([res[c]["out"] for c in range(NCORES)], axis=0)
    return np.ascontiguousarray(out.reshape(B, S, D))
